# revision 22
# baseline (speedup 1.0000x reference)
"""ContraNorm (NormLayer 'CN' branch) on 8 Trainium2 NeuronCores — v5.

kernel(x, adj) -> (1+s)*x - s * softmax(mask(cossim(x, x))) @ x  with s=1.

v5 strategy (vs v2 fp8 baseline, 356us):
Serialization sources identified from the cost-model timeline + the PE
microarch docs:
1. Every dma_start occupies the single global HWDGE dispatcher ~625ns
   regardless of size; v2 issued ~420 DMAs/iter (~260us of pure HWDGE
   serialization).  v5 pre-arranges every DRAM operand host-side into its
   exact SBUF per-partition layout, so an iteration needs ~30 large DMAs
   (~20us of HWDGE).
2. The DVE mask-multiply (e*adj, ~142us DVE-busy in v2) is replaced by a
   PE-side mask: a third DoubleRow matmul with stationary
   [-224*I | -224*I] fp8 against a mask tile holding 224.0 at masked
   entries adds -224*224*2 = -100352 to sim inside the QK PSUM
   accumulation; exp then underflows to exactly 0.  (224, not 448: host
   ml_dtypes float8_e4m3 is the IEEE variant with max finite 240; 448
   quantizes to inf and 0*inf = NaN in the PE.)
3. exp (ACT) writes p2 fp8 directly; DVE only does the tiny finalize.
4. Every matmul uses free-dim 512 (query blocks of 512): DoubleRow
   LDWEIGHTS costs ~210ns (256 columns, FWL off) and only pipelines
   behind a long-enough moving stream — measured production spacing is
   ~131ns/MM at FD=512 vs ~load-bound at FD=256.
Per query block: for each key-tile pair, QK(2 tiles x 2 DR passes) +
2 mask passes -> exp per tile -> PV+den for the previous pair (PE lag 1
pair keeps the in-order PE queue from parking on the ACT chain).
PSUM: 3 rotating sim banks + 4 PV accumulators + den = 8 banks.
Predicted engine busy/iter: PE ~140us, ACT ~122us, DMA ~75us, HWDGE 20us.
"""
import os
import sys

sys.path.insert(0, '/opt/trn_rl_repo')

from contextlib import ExitStack

import numpy as np
import ml_dtypes

import concourse.bass as bass
import concourse.tile as tile
from concourse import mybir

F32 = mybir.dt.float32
BF16 = mybir.dt.bfloat16
F8 = mybir.dt.float8e4
AF = mybir.ActivationFunctionType
ALU = mybir.AluOpType
DR = mybir.MatmulPerfMode.DoubleRow

N = 10000
D = 512
N_CORES = 8
N_PAD = 10240            # 80 key tiles of 128; == 8 * 1280
NQ = N_PAD // N_CORES    # 1280 query rows per core (core 7: 1040 real)
QSCALE = 16.0            # normalized rows scaled by 16 before fp8 quant
MASKV = 224.0            # fp8 mask value at masked entries (<=240!)
IDENV = -224.0           # identity stationary; 2 * (-224 * 224) = -100352
QBW = 512                # query block width (PSUM bank = 512 f32)


def _qblocks(nq):
    out = []
    q0 = 0
    while q0 < nq:
        qsz = min(QBW, nq - q0)
        out.append((q0, qsz))
        q0 += qsz
    return out


def _split_excess_waits(nc, max_waits=1):
    """Walrus CoreV3 rejects >1 sync wait per CTRL instruction; Tile's tail
    drain carries one wait per outstanding engine/DMA queue.  Hoist monotone
    (sem-ge) waits onto same-engine NoOps placed immediately before the
    offending instruction — semantically identical, since the engine executes
    them in program order."""
    for f in nc.m.functions:
        for bb in f.blocks:
            insts = list(bb.instructions)
            new_insts = []
            changed = False
            for inst in insts:
                si = inst.sync_info
                waits = list(si.on_wait) if si is not None else []
                if len(waits) > max_waits:
                    ge = [w for w in waits if "eq" not in (w.wait_mode or "")]
                    eq = [w for w in waits if "eq" in (w.wait_mode or "")]
                    keep_n = max(max_waits - len(eq), 0)
                    n_extra = max(len(ge) - keep_n, 0)
                    extra, keep = ge[:n_extra], ge[n_extra:] + eq
                    if len(keep) > max_waits:
                        raise RuntimeError(
                            f"{inst.name}: non-monotone waits exceed limit")
                    for ci in range(0, len(extra), max_waits):
                        nop = mybir.InstNoOp(
                            name=f"{inst.name}_waitc{ci}",
                            engine=inst.engine,
                            bass_nofuse=True,
                            sync_info=mybir.SyncInfo(
                                on_wait=extra[ci:ci + max_waits], on_update=[]),
                        )
                        new_insts.append(nop)
                    si.on_wait = keep
                    inst.sync_info = si
                    changed = True
                new_insts.append(inst)
            if changed:
                bb.instructions = new_insts


def build(N_pad=N_PAD, NQ_=NQ, D_=D, R=1, drainfix=True, n_cores=N_CORES):
    KT = N_pad // 128        # key tiles
    DT = D_ // 128           # contraction 128-blocks (4)
    NP = KT // 2             # key-tile pairs
    qblocks = _qblocks(NQ_)
    NQB = len(qblocks)
    assert KT % 8 == 0 and DT == 4
    # flat per-partition element offsets for the qb-blocked xq/out layout
    xq_off = np.cumsum(
        [0] + [((qsz + 127) // 128) * D_ for _, qsz in qblocks]).tolist()

    nc = bass.Bass("TRN2", target_bir_lowering=False, debug=False,
                   num_devices=n_cores)
    # All DRAM operands pre-arranged host-side to the SBUF per-partition
    # layout, so each loads with a single large-element DMA.
    xb2 = nc.declare_dram_parameter("xb2", [128, KT * D_], F8, isOutput=False)
    xkT2 = nc.declare_dram_parameter("xkT2", [128, DT * N_pad], F8,
                                     isOutput=False)
    qnT2 = nc.declare_dram_parameter("qnT2", [128, DT * NQ_], F8,
                                     isOutput=False)
    mkTq = nc.declare_dram_parameter("mkTq", [NQB, 128, KT * QBW], F8,
                                     isOutput=False)
    iden = nc.declare_dram_parameter("iden", [128, 256], F8, isOutput=False)
    xq2b = nc.declare_dram_parameter("xq2b", [128, xq_off[-1]], BF16,
                                     isOutput=False)
    out2 = nc.declare_dram_parameter("out2", [128, xq_off[-1]], BF16,
                                     isOutput=True)

    with tile.TileContext(nc) as tc, ExitStack() as ctx:
        resident = ctx.enter_context(tc.tile_pool(name="resident", bufs=1))
        small = ctx.enter_context(tc.tile_pool(name="small", bufs=1))
        m_pool = ctx.enter_context(tc.tile_pool(name="maskp", bufs=2))
        p_pool = ctx.enter_context(tc.tile_pool(name="pp", bufs=4))
        xq_pool = ctx.enter_context(tc.tile_pool(name="xqp", bufs=2))
        fin_pool = ctx.enter_context(tc.tile_pool(name="finp", bufs=2))
        sim_psum = ctx.enter_context(
            tc.tile_pool(name="simps", bufs=3, space="PSUM"))
        acc_psum = ctx.enter_context(
            tc.tile_pool(name="accps", bufs=1, space="PSUM"))

        def body(_i=None):
            xb_s = resident.tile([128, KT, D_], F8, tag="xb_s")
            xkT_s = resident.tile([128, DT, N_pad], F8, tag="xkT_s")
            qnT_s = resident.tile([128, DT, NQ_], F8, tag="qnT_s")
            iden_s = resident.tile([128, 2, 128], F8, tag="iden_s")

            onesw = small.tile([128, 2, 16], F8, tag="onesw")
            nc.vector.memset(onesw, 1.0)
            onef = small.tile([128, 16], F32, tag="onef")
            nc.vector.memset(onef, 1.0)
            den_s = small.tile([1, 512], F32, tag="den_s")

            def load_masks(m_sb, qb, qsz, nsplit=2):
                # masks stored 512-wide per tile; only [:qsz] columns read
                kc = KT // nsplit
                for c in range(nsplit):
                    nc.sync.dma_start(
                        out=m_sb[:, c * kc:(c + 1) * kc, :],
                        in_=mkTq[qb, :, c * kc * QBW:(c + 1) * kc * QBW])

            # ---- prologue DMAs (all operands; few large DMAs) ----
            nc.sync.dma_start(out=iden_s[:, :, :], in_=iden[:, :])
            for dt in range(DT):
                nc.sync.dma_start(out=qnT_s[:, dt, :],
                                  in_=qnT2[:, dt * NQ_:(dt + 1) * NQ_])
            m_sb = m_pool.tile([128, KT, QBW], F8, tag="m_sb")
            for dt in range(DT):
                nc.sync.dma_start(
                    out=xkT_s[:, dt, :],
                    in_=xkT2[:, dt * N_pad:(dt + 1) * N_pad])
                if dt == 1:
                    load_masks(m_sb, 0, qblocks[0][1], nsplit=2)
            XBC = 8
            for c in range(XBC):
                k0 = c * (KT // XBC)
                k1 = (c + 1) * (KT // XBC)
                nc.sync.dma_start(out=xb_s[:, k0:k1, :],
                                  in_=xb2[:, k0 * D_:k1 * D_])

            def emit_pv(p2W, pi, qsz, nj, den, outp):
                st = (pi == 0)
                sp = (pi == NP - 1)
                for j in range(nj):
                    nc.tensor.matmul(
                        outp[j][:, :],
                        lhsT=p2W[:, :, j * 128:(j + 1) * 128],
                        rhs=xb_s[:, 2 * pi:2 * pi + 2, :],
                        start=st, stop=sp, perf_mode=DR)
                nc.tensor.matmul(
                    den[0:1, :qsz], lhsT=onesw[:, :, 0:1],
                    rhs=p2W[:, :, :qsz],
                    start=st, stop=sp, perf_mode=DR,
                    skip_group_check=True)

            # ---- main loop over query blocks ----
            for qb, (q0, qsz) in enumerate(qblocks):
                nj = (qsz + 127) // 128
                outp = [acc_psum.tile([128, D_], F32, tag=f"outp{j}",
                                      name=f"outp{j}") for j in range(nj)]
                den = acc_psum.tile([128, 512], F32, tag="den")
                m_nxt = None
                if qb + 1 < NQB:
                    m_nxt = m_pool.tile([128, KT, QBW], F8, tag="m_sb",
                                        name="m_nxt")
                xq_f = xq_pool.tile([128, 4, D_], BF16, tag="xq_f")
                pvq = []  # queued (p2W, pair, qsz, nj); PV lags QK by 2 pairs
                for pi in range(NP):
                    p2W = p_pool.tile([128, 2, QBW], F8, tag="p2W")
                    simWs = []
                    for i in range(2):
                        kt = 2 * pi + i
                        simW = sim_psum.tile([128, QBW], F32, tag="simW")
                        simWs.append(simW)
                        nc.tensor.matmul(
                            simW[:, :qsz],
                            lhsT=xkT_s[:, 0:2, kt * 128:(kt + 1) * 128],
                            rhs=qnT_s[:, 0:2, q0:q0 + qsz],
                            start=True, stop=False, perf_mode=DR)
                        nc.tensor.matmul(
                            simW[:, :qsz],
                            lhsT=xkT_s[:, 2:4, kt * 128:(kt + 1) * 128],
                            rhs=qnT_s[:, 2:4, q0:q0 + qsz],
                            start=False, stop=False, perf_mode=DR)
                    for i in range(2):
                        kt = 2 * pi + i
                        m_rhs = m_sb[:, kt, :qsz][:, None, :].broadcast_to(
                            [128, 2, qsz])
                        nc.tensor.matmul(
                            simWs[i][:, :qsz], lhsT=iden_s[:, :, :],
                            rhs=m_rhs,
                            start=False, stop=True, perf_mode=DR)
                    for i in range(2):
                        nc.scalar.activation(out=p2W[:, i, :qsz],
                                             in_=simWs[i][:, :qsz],
                                             func=AF.Exp,
                                             scale=1.0 / (QSCALE * QSCALE))
                    if pi == 0:
                        nc.sync.dma_start(
                            out=xq_f[:, 0:nj, :],
                            in_=xq2b[:, xq_off[qb]:xq_off[qb + 1]])
                        if m_nxt is not None:
                            load_masks(m_nxt, qb + 1, qblocks[qb + 1][1])
                    if len(pvq) == 2:
                        emit_pv(*pvq.pop(0), den, outp)
                    pvq.append((p2W, pi, qsz, nj))
                for e in pvq:
                    emit_pv(*e, den, outp)
                if m_nxt is not None:
                    m_sb = m_nxt

                # ---- finalize: out = 2*xq - outp/den ----
                nc.vector.tensor_copy(out=den_s[0:1, :qsz], in_=den[0:1, :qsz])
                tps = sim_psum.tile([128, QBW], F32, tag="simW", name="tps")
                o_t2 = fin_pool.tile([128, 4, D_], BF16, tag="o_t2")
                for j in range(nj):
                    nc.tensor.matmul(tps[:, j:j + 1],
                                     lhsT=den_s[0:1, j * 128:(j + 1) * 128],
                                     rhs=onef[0:1, 0:1],
                                     start=True, stop=True,
                                     skip_group_check=True)
                    rden = fin_pool.tile([128, 1], F32, tag="rden")
                    nc.vector.reciprocal(out=rden[:, :], in_=tps[:, j:j + 1])
                    t1 = fin_pool.tile([128, D_], F32, tag="t1")
                    nc.vector.tensor_scalar_mul(out=t1[:, :],
                                                in0=outp[j][:, :],
                                                scalar1=rden[:, :])
                    nc.vector.tensor_tensor(out=o_t2[:, j, :],
                                            in0=xq_f[:, j, :],
                                            in1=t1[:, :], op=ALU.subtract)
                nc.sync.dma_start(
                    out=out2[:, xq_off[qb]:xq_off[qb + 1]],
                    in_=o_t2[:, 0:nj, :])

        if R == 1:
            body()
        else:
            with tc.For_i(0, R, 1) as i:
                body(i)

    if drainfix:
        _split_excess_waits(nc, 1)
    return nc


def prep_inputs(x, adj, n_pad=N_PAD, nq=NQ, n_cores=N_CORES, n_real=N):
    """Host-side shard/layout prep. Returns in_maps for run_bass_kernel_spmd.

    Every operand is pre-arranged into the kernel's SBUF per-partition
    layout so each loads with a single large-element DMA."""
    f8 = ml_dtypes.float8_e4m3
    x = np.asarray(x, dtype=np.float32)
    d = x.shape[1]
    KT = n_pad // 128
    DT = d // 128
    qblocks = _qblocks(nq)
    NQB = len(qblocks)
    rn = np.maximum(np.linalg.norm(x, axis=1, keepdims=True), 1e-12)
    nx = (QSCALE / rn) * x                       # 16 * x/|x|
    xb = np.zeros((n_pad, d), dtype=f8)
    xb[:n_real] = x.astype(f8)
    # xb2[p, kt*D + c] = xb[kt*128 + p, c]
    xb2 = np.ascontiguousarray(
        xb.reshape(KT, 128, d).transpose(1, 0, 2).reshape(128, KT * d))
    xk = np.zeros((n_pad, d), dtype=np.float32)
    xk[:n_real] = nx
    xkT = np.ascontiguousarray(xk.T).astype(f8)      # [D, n_pad]
    # xkT2[p, dt*n_pad + k] = xkT[dt*128 + p, k]
    xkT2 = np.ascontiguousarray(
        xkT.reshape(DT, 128, n_pad).transpose(1, 0, 2).reshape(128, -1))
    iden = np.zeros((128, 256), dtype=f8)
    ii = np.arange(128)
    iden[ii, ii] = IDENV
    iden[ii, 128 + ii] = IDENV
    in_maps = []
    for c in range(n_cores):
        q0c = c * nq
        q1c = min(q0c + nq, n_real)
        nreal = max(q1c - q0c, 0)
        mkT_c = np.full((n_pad, nq), MASKV, dtype=f8)
        if nreal > 0:
            mkT_c[:n_real, :nreal] = np.where(
                adj[q0c:q1c, :].T > 0, MASKV, 0.0).astype(f8)
        # mkTq[qb, p, kt*QBW + cq] = mkT_c[kt*128 + p, q0 + cq], 224-padded
        mkTq = np.full((NQB, 128, KT * QBW), MASKV, dtype=f8)
        for qb, (q0, qsz) in enumerate(qblocks):
            blk = mkT_c[:, q0:q0 + qsz].reshape(KT, 128, qsz)
            mkTq[qb].reshape(128, KT, QBW)[:, :, :qsz] = \
                blk.transpose(1, 0, 2)
        qnT_c = np.zeros((d, nq), dtype=np.float32)
        xq2_c = np.zeros((nq, d), dtype=np.float32)
        if nreal > 0:
            qnT_c[:, :nreal] = nx[q0c:q1c].T
            xq2_c[:nreal] = 2.0 * x[q0c:q1c]
        # qnT2[p, dt*nq + q] = qnT_c[dt*128 + p, q]
        qnT2 = np.ascontiguousarray(
            qnT_c.astype(f8).reshape(DT, 128, nq).transpose(1, 0, 2)
            .reshape(128, -1))
        # xq2b[p, xq_off[qb] + j*D + cd] = 2*x[q0c + q0 + j*128 + p, cd]
        xqb = xq2_c.astype(ml_dtypes.bfloat16)
        xq_parts = []
        for q0, qsz in qblocks:
            nj = (qsz + 127) // 128
            xq_parts.append(xqb[q0:q0 + qsz].reshape(nj, 128, d)
                            .transpose(1, 0, 2).reshape(128, -1))
        xq2b = np.ascontiguousarray(np.concatenate(xq_parts, axis=1))
        in_maps.append({"xb2": xb2, "xkT2": xkT2, "qnT2": qnT2,
                        "mkTq": mkTq, "iden": iden, "xq2b": xq2b})
    return in_maps


def unshuffle_out(out2, nq=NQ, d=D):
    """out2 [128, sum_j D] bf16 -> [NQ, D] f32 (row q0 + j*128 + p)."""
    o = np.asarray(out2, dtype=np.float32)
    rows = []
    off = 0
    for q0, qsz in _qblocks(nq):
        nj = (qsz + 127) // 128
        blk = o[:, off:off + nj * d].reshape(128, nj, d)
        rows.append(blk.transpose(1, 0, 2).reshape(nj * 128, d))
        off += nj * d
    return np.ascontiguousarray(np.concatenate(rows, axis=0)[:nq])


_cached = {}


def _get_nc(R=1):
    if R not in _cached:
        _cached[R] = build(R=R)
    return _cached[R]


_neff_cache_installed = False


def _install_neff_cache():
    """Disk-cache walrus NEFF compiles keyed by the BIR JSON hash, so repeat
    processes skip the multi-minute compile."""
    global _neff_cache_installed
    if _neff_cache_installed:
        return
    _neff_cache_installed = True
    import hashlib
    import shutil
    from concourse import bass2jax
    cache_dir = os.path.expanduser("~/.cache/bass_neff_cache")
    os.makedirs(cache_dir, exist_ok=True)
    orig = bass2jax.compile_bir_kernel

    def cached(bir_json, tmpdir, neff_name="file.neff"):
        key = hashlib.sha256(
            bir_json if isinstance(bir_json, bytes) else bir_json.encode()
        ).hexdigest()[:32]
        hit = os.path.join(cache_dir, key + ".neff")
        dst = os.path.join(tmpdir, neff_name)
        if os.path.exists(hit):
            shutil.copyfile(hit, dst)
            return dst
        path = orig(bir_json, tmpdir, neff_name)
        try:
            shutil.copyfile(path, hit)
        except OSError:
            pass
        return path

    bass2jax.compile_bir_kernel = cached


def run_on_cores(in_maps, R=1):
    _install_neff_cache()
    from concourse.bass_utils import run_bass_kernel_spmd
    nc = _get_nc(R)
    res = run_bass_kernel_spmd(nc, in_maps, list(range(N_CORES)))
    return [res.results[c]["out2"] for c in range(N_CORES)]


def kernel(x, adj):
    x = np.asarray(x, dtype=np.float32)
    adj = np.asarray(adj, dtype=np.int32)
    assert x.shape == (N, D) and adj.shape == (N, N)
    in_maps = prep_inputs(x, adj)
    outs = run_on_cores(in_maps, R=1)
    full = np.concatenate([unshuffle_out(o) for o in outs], axis=0)[:N]
    return np.ascontiguousarray(full.astype(np.float32))


# revision 23
# speedup vs baseline: 1.2531x; 1.2531x over previous
"""ContraNorm (NormLayer 'CN' branch) on 8 Trainium2 NeuronCores — v5.

kernel(x, adj) -> (1+s)*x - s * softmax(mask(cossim(x, x))) @ x  with s=1.

v5 strategy (vs v2 fp8 baseline, 356us):
Serialization sources identified from the cost-model timeline + the PE
microarch docs:
1. Every dma_start occupies the single global HWDGE dispatcher ~625ns
   regardless of size; v2 issued ~420 DMAs/iter (~260us of pure HWDGE
   serialization).  v5 pre-arranges every DRAM operand host-side into its
   exact SBUF per-partition layout, so an iteration needs ~30 large DMAs
   (~20us of HWDGE).
2. The DVE mask-multiply (e*adj, ~142us DVE-busy in v2) is replaced by a
   PE-side mask: a third DoubleRow matmul with stationary
   [-224*I | -224*I] fp8 against a mask tile holding 224.0 at masked
   entries adds -224*224*2 = -100352 to sim inside the QK PSUM
   accumulation; exp then underflows to exactly 0.  (224, not 448: host
   ml_dtypes float8_e4m3 is the IEEE variant with max finite 240; 448
   quantizes to inf and 0*inf = NaN in the PE.)
3. exp (ACT) writes p2 fp8 directly; DVE only does the tiny finalize.
4. Every matmul uses free-dim 512 (query blocks of 512): DoubleRow
   LDWEIGHTS costs ~210ns (256 columns, FWL off) and only pipelines
   behind a long-enough moving stream — measured production spacing is
   ~131ns/MM at FD=512 vs ~load-bound at FD=256.
Per query block: for each key-tile pair, QK(2 tiles x 2 DR passes) +
2 mask passes -> exp per tile -> PV+den for the previous pair (PE lag 1
pair keeps the in-order PE queue from parking on the ACT chain).
PSUM: 3 rotating sim banks + 4 PV accumulators + den = 8 banks.
Predicted engine busy/iter: PE ~140us, ACT ~122us, DMA ~75us, HWDGE 20us.
"""
import os
import sys

sys.path.insert(0, '/opt/trn_rl_repo')

from contextlib import ExitStack

import numpy as np
import ml_dtypes

import concourse.bass as bass
import concourse.tile as tile
from concourse import mybir

F32 = mybir.dt.float32
BF16 = mybir.dt.bfloat16
F8 = mybir.dt.float8e4
AF = mybir.ActivationFunctionType
ALU = mybir.AluOpType
DR = mybir.MatmulPerfMode.DoubleRow

N = 10000
D = 512
N_CORES = 8
N_PAD = 10240            # 80 key tiles of 128; == 8 * 1280
NQ = N_PAD // N_CORES    # 1280 query rows per core (core 7: 1040 real)
QSCALE = 16.0            # normalized rows scaled by 16 before fp8 quant
MASKV = 224.0            # fp8 mask value at masked entries (<=240!)
IDENV = -224.0           # identity stationary; 2 * (-224 * 224) = -100352
QBW = 512                # query block width (PSUM bank = 512 f32)


def _qblocks(nq):
    out = []
    q0 = 0
    while q0 < nq:
        qsz = min(QBW, nq - q0)
        out.append((q0, qsz))
        q0 += qsz
    return out


def _split_excess_waits(nc, max_waits=1):
    """Walrus CoreV3 rejects >1 sync wait per CTRL instruction; Tile's tail
    drain carries one wait per outstanding engine/DMA queue.  Hoist monotone
    (sem-ge) waits onto same-engine NoOps placed immediately before the
    offending instruction — semantically identical, since the engine executes
    them in program order."""
    for f in nc.m.functions:
        for bb in f.blocks:
            insts = list(bb.instructions)
            new_insts = []
            changed = False
            for inst in insts:
                si = inst.sync_info
                waits = list(si.on_wait) if si is not None else []
                if len(waits) > max_waits:
                    ge = [w for w in waits if "eq" not in (w.wait_mode or "")]
                    eq = [w for w in waits if "eq" in (w.wait_mode or "")]
                    keep_n = max(max_waits - len(eq), 0)
                    n_extra = max(len(ge) - keep_n, 0)
                    extra, keep = ge[:n_extra], ge[n_extra:] + eq
                    if len(keep) > max_waits:
                        raise RuntimeError(
                            f"{inst.name}: non-monotone waits exceed limit")
                    for ci in range(0, len(extra), max_waits):
                        nop = mybir.InstNoOp(
                            name=f"{inst.name}_waitc{ci}",
                            engine=inst.engine,
                            bass_nofuse=True,
                            sync_info=mybir.SyncInfo(
                                on_wait=extra[ci:ci + max_waits], on_update=[]),
                        )
                        new_insts.append(nop)
                    si.on_wait = keep
                    inst.sync_info = si
                    changed = True
                new_insts.append(inst)
            if changed:
                bb.instructions = new_insts


def build(N_pad=N_PAD, NQ_=NQ, D_=D, R=1, drainfix=True, n_cores=N_CORES):
    KT = N_pad // 128        # key tiles
    DT = D_ // 128           # contraction 128-blocks (4)
    NP = KT // 2             # key-tile pairs
    qblocks = _qblocks(NQ_)
    NQB = len(qblocks)
    assert KT % 8 == 0 and DT == 4
    # flat per-partition element offsets for the qb-blocked xq/out layout
    xq_off = np.cumsum(
        [0] + [((qsz + 127) // 128) * D_ for _, qsz in qblocks]).tolist()

    nc = bass.Bass("TRN2", target_bir_lowering=False, debug=False,
                   num_devices=n_cores)
    # All DRAM operands pre-arranged host-side to the SBUF per-partition
    # layout, so each loads with a single large-element DMA.
    xb2 = nc.declare_dram_parameter("xb2", [128, KT * D_], F8, isOutput=False)
    xkT2 = nc.declare_dram_parameter("xkT2", [128, DT * N_pad], F8,
                                     isOutput=False)
    qnT2 = nc.declare_dram_parameter("qnT2", [128, DT * NQ_], F8,
                                     isOutput=False)
    mkTq = nc.declare_dram_parameter("mkTq", [NQB, 128, KT * QBW], F8,
                                     isOutput=False)
    iden = nc.declare_dram_parameter("iden", [128, 256], F8, isOutput=False)
    xq2b = nc.declare_dram_parameter("xq2b", [128, xq_off[-1]], BF16,
                                     isOutput=False)
    out2 = nc.declare_dram_parameter("out2", [128, xq_off[-1]], BF16,
                                     isOutput=True)

    with tile.TileContext(nc) as tc, ExitStack() as ctx:
        resident = ctx.enter_context(tc.tile_pool(name="resident", bufs=1))
        small = ctx.enter_context(tc.tile_pool(name="small", bufs=1))
        m_pool = ctx.enter_context(tc.tile_pool(name="maskp", bufs=2))
        p_pool = ctx.enter_context(tc.tile_pool(name="pp", bufs=3))
        xq_pool = ctx.enter_context(tc.tile_pool(name="xqp", bufs=2))
        fin_pool = ctx.enter_context(tc.tile_pool(name="finp", bufs=2))
        sim_psum = ctx.enter_context(
            tc.tile_pool(name="simps", bufs=3, space="PSUM"))
        acc_psum = ctx.enter_context(
            tc.tile_pool(name="accps", bufs=1, space="PSUM"))

        def body(_i=None):
            xb_s = resident.tile([128, KT, D_], F8, tag="xb_s")
            xkT_s = resident.tile([128, DT, N_pad], F8, tag="xkT_s")
            qnT_s = resident.tile([128, DT, NQ_], F8, tag="qnT_s")
            iden_s = resident.tile([128, 2, 128], F8, tag="iden_s")

            onesw = small.tile([128, 2, 16], F8, tag="onesw")
            nc.vector.memset(onesw, 1.0)
            onef = small.tile([128, 16], F32, tag="onef")
            nc.vector.memset(onef, 1.0)
            den_s = small.tile([1, 512], F32, tag="den_s")

            def load_masks(m_sb, qb, qsz, nsplit=2):
                # masks stored 512-wide per tile; only [:qsz] columns read
                kc = KT // nsplit
                for c in range(nsplit):
                    nc.sync.dma_start(
                        out=m_sb[:, c * kc:(c + 1) * kc, :],
                        in_=mkTq[qb, :, c * kc * QBW:(c + 1) * kc * QBW])

            # ---- prologue DMAs (all operands; few large DMAs) ----
            nc.sync.dma_start(out=iden_s[:, :, :], in_=iden[:, :])
            for dt in range(DT):
                nc.sync.dma_start(out=qnT_s[:, dt, :],
                                  in_=qnT2[:, dt * NQ_:(dt + 1) * NQ_])
            m_sb = m_pool.tile([128, KT, QBW], F8, tag="m_sb")
            for dt in range(DT):
                nc.sync.dma_start(
                    out=xkT_s[:, dt, :],
                    in_=xkT2[:, dt * N_pad:(dt + 1) * N_pad])
                if dt == 1:
                    load_masks(m_sb, 0, qblocks[0][1], nsplit=2)
            XBC = 8
            for c in range(XBC):
                k0 = c * (KT // XBC)
                k1 = (c + 1) * (KT // XBC)
                nc.sync.dma_start(out=xb_s[:, k0:k1, :],
                                  in_=xb2[:, k0 * D_:k1 * D_])

            def emit_pv(p2W, pi, qsz, nj, den, outp):
                st = (pi == 0)
                sp = (pi == NP - 1)
                for j in range(nj):
                    nc.tensor.matmul(
                        outp[j][:, :],
                        lhsT=p2W[:, :, j * 128:(j + 1) * 128],
                        rhs=xb_s[:, 2 * pi:2 * pi + 2, :],
                        start=st, stop=sp, perf_mode=DR)
                nc.tensor.matmul(
                    den[0:1, :qsz], lhsT=onesw[:, :, 0:1],
                    rhs=p2W[:, :, :qsz],
                    start=st, stop=sp, perf_mode=DR,
                    skip_group_check=True)

            # ---- main loop over query blocks ----
            for qb, (q0, qsz) in enumerate(qblocks):
                nj = (qsz + 127) // 128
                outp = [acc_psum.tile([128, D_], F32, tag=f"outp{j}",
                                      name=f"outp{j}") for j in range(nj)]
                den = acc_psum.tile([128, 512], F32, tag="den")
                m_nxt = None
                if qb + 1 < NQB:
                    m_nxt = m_pool.tile([128, KT, QBW], F8, tag="m_sb",
                                        name="m_nxt")
                xq_f = xq_pool.tile([128, 4, D_], BF16, tag="xq_f")
                prev = None  # (p2W, pair, qsz, nj)
                for pi in range(NP):
                    p2W = p_pool.tile([128, 2, QBW], F8, tag="p2W")
                    simWs = []
                    for i in range(2):
                        kt = 2 * pi + i
                        simW = sim_psum.tile([128, QBW], F32, tag="simW")
                        simWs.append(simW)
                        nc.tensor.matmul(
                            simW[:, :qsz],
                            lhsT=xkT_s[:, 0:2, kt * 128:(kt + 1) * 128],
                            rhs=qnT_s[:, 0:2, q0:q0 + qsz],
                            start=True, stop=False, perf_mode=DR)
                        nc.tensor.matmul(
                            simW[:, :qsz],
                            lhsT=xkT_s[:, 2:4, kt * 128:(kt + 1) * 128],
                            rhs=qnT_s[:, 2:4, q0:q0 + qsz],
                            start=False, stop=False, perf_mode=DR)
                    for i in range(2):
                        kt = 2 * pi + i
                        m_rhs = m_sb[:, kt, :qsz][:, None, :].broadcast_to(
                            [128, 2, qsz])
                        nc.tensor.matmul(
                            simWs[i][:, :qsz], lhsT=iden_s[:, :, :],
                            rhs=m_rhs,
                            start=False, stop=True, perf_mode=DR)
                    for i in range(2):
                        nc.scalar.activation(out=p2W[:, i, :qsz],
                                             in_=simWs[i][:, :qsz],
                                             func=AF.Exp,
                                             scale=1.0 / (QSCALE * QSCALE))
                    if pi == 0:
                        nc.sync.dma_start(
                            out=xq_f[:, 0:nj, :],
                            in_=xq2b[:, xq_off[qb]:xq_off[qb + 1]])
                        if m_nxt is not None:
                            load_masks(m_nxt, qb + 1, qblocks[qb + 1][1])
                    if prev is not None:
                        emit_pv(*prev, den, outp)
                    prev = (p2W, pi, qsz, nj)
                emit_pv(*prev, den, outp)
                if m_nxt is not None:
                    m_sb = m_nxt

                # ---- finalize: out = 2*xq - outp/den ----
                nc.vector.tensor_copy(out=den_s[0:1, :qsz], in_=den[0:1, :qsz])
                tps = sim_psum.tile([128, QBW], F32, tag="simW", name="tps")
                o_t2 = fin_pool.tile([128, 4, D_], BF16, tag="o_t2")
                for j in range(nj):
                    nc.tensor.matmul(tps[:, j:j + 1],
                                     lhsT=den_s[0:1, j * 128:(j + 1) * 128],
                                     rhs=onef[0:1, 0:1],
                                     start=True, stop=True,
                                     skip_group_check=True)
                    rden = fin_pool.tile([128, 1], F32, tag="rden")
                    nc.vector.reciprocal(out=rden[:, :], in_=tps[:, j:j + 1])
                    t1 = fin_pool.tile([128, D_], F32, tag="t1")
                    nc.vector.tensor_scalar_mul(out=t1[:, :],
                                                in0=outp[j][:, :],
                                                scalar1=rden[:, :])
                    nc.vector.tensor_tensor(out=o_t2[:, j, :],
                                            in0=xq_f[:, j, :],
                                            in1=t1[:, :], op=ALU.subtract)
                nc.sync.dma_start(
                    out=out2[:, xq_off[qb]:xq_off[qb + 1]],
                    in_=o_t2[:, 0:nj, :])

        if R == 1:
            body()
        else:
            with tc.For_i(0, R, 1) as i:
                body(i)

    if drainfix:
        _split_excess_waits(nc, 1)
    return nc


def prep_inputs(x, adj, n_pad=N_PAD, nq=NQ, n_cores=N_CORES, n_real=N):
    """Host-side shard/layout prep. Returns in_maps for run_bass_kernel_spmd.

    Every operand is pre-arranged into the kernel's SBUF per-partition
    layout so each loads with a single large-element DMA."""
    f8 = ml_dtypes.float8_e4m3
    x = np.asarray(x, dtype=np.float32)
    d = x.shape[1]
    KT = n_pad // 128
    DT = d // 128
    qblocks = _qblocks(nq)
    NQB = len(qblocks)
    rn = np.maximum(np.linalg.norm(x, axis=1, keepdims=True), 1e-12)
    nx = (QSCALE / rn) * x                       # 16 * x/|x|
    xb = np.zeros((n_pad, d), dtype=f8)
    xb[:n_real] = x.astype(f8)
    # xb2[p, kt*D + c] = xb[kt*128 + p, c]
    xb2 = np.ascontiguousarray(
        xb.reshape(KT, 128, d).transpose(1, 0, 2).reshape(128, KT * d))
    xk = np.zeros((n_pad, d), dtype=np.float32)
    xk[:n_real] = nx
    xkT = np.ascontiguousarray(xk.T).astype(f8)      # [D, n_pad]
    # xkT2[p, dt*n_pad + k] = xkT[dt*128 + p, k]
    xkT2 = np.ascontiguousarray(
        xkT.reshape(DT, 128, n_pad).transpose(1, 0, 2).reshape(128, -1))
    iden = np.zeros((128, 256), dtype=f8)
    ii = np.arange(128)
    iden[ii, ii] = IDENV
    iden[ii, 128 + ii] = IDENV
    in_maps = []
    for c in range(n_cores):
        q0c = c * nq
        q1c = min(q0c + nq, n_real)
        nreal = max(q1c - q0c, 0)
        mkT_c = np.full((n_pad, nq), MASKV, dtype=f8)
        if nreal > 0:
            mkT_c[:n_real, :nreal] = np.where(
                adj[q0c:q1c, :].T > 0, MASKV, 0.0).astype(f8)
        # mkTq[qb, p, kt*QBW + cq] = mkT_c[kt*128 + p, q0 + cq], 224-padded
        mkTq = np.full((NQB, 128, KT * QBW), MASKV, dtype=f8)
        for qb, (q0, qsz) in enumerate(qblocks):
            blk = mkT_c[:, q0:q0 + qsz].reshape(KT, 128, qsz)
            mkTq[qb].reshape(128, KT, QBW)[:, :, :qsz] = \
                blk.transpose(1, 0, 2)
        qnT_c = np.zeros((d, nq), dtype=np.float32)
        xq2_c = np.zeros((nq, d), dtype=np.float32)
        if nreal > 0:
            qnT_c[:, :nreal] = nx[q0c:q1c].T
            xq2_c[:nreal] = 2.0 * x[q0c:q1c]
        # qnT2[p, dt*nq + q] = qnT_c[dt*128 + p, q]
        qnT2 = np.ascontiguousarray(
            qnT_c.astype(f8).reshape(DT, 128, nq).transpose(1, 0, 2)
            .reshape(128, -1))
        # xq2b[p, xq_off[qb] + j*D + cd] = 2*x[q0c + q0 + j*128 + p, cd]
        xqb = xq2_c.astype(ml_dtypes.bfloat16)
        xq_parts = []
        for q0, qsz in qblocks:
            nj = (qsz + 127) // 128
            xq_parts.append(xqb[q0:q0 + qsz].reshape(nj, 128, d)
                            .transpose(1, 0, 2).reshape(128, -1))
        xq2b = np.ascontiguousarray(np.concatenate(xq_parts, axis=1))
        in_maps.append({"xb2": xb2, "xkT2": xkT2, "qnT2": qnT2,
                        "mkTq": mkTq, "iden": iden, "xq2b": xq2b})
    return in_maps


def unshuffle_out(out2, nq=NQ, d=D):
    """out2 [128, sum_j D] bf16 -> [NQ, D] f32 (row q0 + j*128 + p)."""
    o = np.asarray(out2, dtype=np.float32)
    rows = []
    off = 0
    for q0, qsz in _qblocks(nq):
        nj = (qsz + 127) // 128
        blk = o[:, off:off + nj * d].reshape(128, nj, d)
        rows.append(blk.transpose(1, 0, 2).reshape(nj * 128, d))
        off += nj * d
    return np.ascontiguousarray(np.concatenate(rows, axis=0)[:nq])


_cached = {}


def _get_nc(R=1):
    if R not in _cached:
        _cached[R] = build(R=R)
    return _cached[R]


_neff_cache_installed = False


def _install_neff_cache():
    """Disk-cache walrus NEFF compiles keyed by the BIR JSON hash, so repeat
    processes skip the multi-minute compile."""
    global _neff_cache_installed
    if _neff_cache_installed:
        return
    _neff_cache_installed = True
    import hashlib
    import shutil
    from concourse import bass2jax
    cache_dir = os.path.expanduser("~/.cache/bass_neff_cache")
    os.makedirs(cache_dir, exist_ok=True)
    orig = bass2jax.compile_bir_kernel

    def cached(bir_json, tmpdir, neff_name="file.neff"):
        key = hashlib.sha256(
            bir_json if isinstance(bir_json, bytes) else bir_json.encode()
        ).hexdigest()[:32]
        hit = os.path.join(cache_dir, key + ".neff")
        dst = os.path.join(tmpdir, neff_name)
        if os.path.exists(hit):
            shutil.copyfile(hit, dst)
            return dst
        path = orig(bir_json, tmpdir, neff_name)
        try:
            shutil.copyfile(path, hit)
        except OSError:
            pass
        return path

    bass2jax.compile_bir_kernel = cached


def run_on_cores(in_maps, R=1):
    _install_neff_cache()
    from concourse.bass_utils import run_bass_kernel_spmd
    nc = _get_nc(R)
    res = run_bass_kernel_spmd(nc, in_maps, list(range(N_CORES)))
    return [res.results[c]["out2"] for c in range(N_CORES)]


def kernel(x, adj):
    x = np.asarray(x, dtype=np.float32)
    adj = np.asarray(adj, dtype=np.int32)
    assert x.shape == (N, D) and adj.shape == (N, N)
    in_maps = prep_inputs(x, adj)
    outs = run_on_cores(in_maps, R=1)
    full = np.concatenate([unshuffle_out(o) for o in outs], axis=0)[:N]
    return np.ascontiguousarray(full.astype(np.float32))
